# revision 33
# baseline (speedup 1.0000x reference)
"""Trainium2 Bass kernel for the AttentionalLaneLSTM problem.

Full-input contract: kernel(**inputs) takes the complete (unsharded) numpy
inputs and returns the full (4096, 128) float32 output. Internally the lane
dimension M=4096 is sharded 8 ways (512 lanes per NeuronCore); all small
weights and the obstacle encoding are replicated so each core computes its
own attention table and gathers its rows locally.

Per-core dataflow (everything feature-major: features on SBUF partitions,
lanes on the free dim so the LSTM recurrence needs no transposes):
  1. scores = softmax(relu(obs @ attn_W)) computed on-chip (256x100).
  2. att^T = scores^T gathered per lane via a one-hot matmul
     (att^T[t,m] = scores[mask[m], t]), staged to DRAM for per-step
     partition-broadcast loads.
  3. lane_features tail (512,400) is transposed on the PE, embedded for two
     timesteps per matmul via a padded block-diagonal embed weight, relu'd
     into a packed x^T buffer (128, 25600) bf16 (even t rows 0:64, odd 64:128).
  4. 100 bidirectional LSTM steps, fwd/bwd chains phase-interleaved.
     Gates accumulate x@Wih + h@Whh in PSUM (8 banks = 2 dirs x [f|i|o]+g),
     sigmoid/tanh on ScalarE, cell update + running max on VectorE with a
     lane-halved tail (c/tanh/h in 256-lane halves so the recurrence tail
     pipelines), attention product + accumulate on GpSimd, per-step
     attention rows broadcast-loaded from a DRAM stage via stride-0 DMA.
  5. Final encoder: 8 accumulating matmuls over [front|back|max|attn]
     feature blocks, relu, PE transpose back to lane-major, DMA out.
"""

import os
import sys

if "/opt/trn_rl_repo" not in sys.path:
    sys.path.insert(0, "/opt/trn_rl_repo")

import numpy as np
import ml_dtypes

import concourse.bass as bass
import concourse.bacc as bacc
import concourse.mybir as mybir
from concourse.tile import TileContext
from concourse.masks import make_identity

BF16 = mybir.dt.bfloat16
F32 = mybir.dt.float32
AF = mybir.ActivationFunctionType
OP = mybir.AluOpType
nbf16 = ml_dtypes.bfloat16

N_CORES = 8
M, N_OBS, EMB, H, ENC, T = 4096, 256, 64, 128, 128, 100
MS = M // N_CORES  # 512 lanes per core


def _build_program():
    nc = bacc.Bacc("TRN2", target_bir_lowering=False, debug=False,
                   num_devices=N_CORES)

    lf_d = nc.declare_dram_parameter("lf", [MS, 400], F32, isOutput=False)
    obs_d = nc.declare_dram_parameter("obs", [N_OBS, H], F32, isOutput=False)
    maskf_d = nc.declare_dram_parameter("maskf", [1, MS], F32, isOutput=False)
    aux_d = nc.declare_dram_parameter("aux", [128, 6], F32, isOutput=False)
    wpad_d = nc.declare_dram_parameter("wpad", [128, 2048], BF16, isOutput=False)
    wihf_d = nc.declare_dram_parameter("wihf", [128, 512], BF16, isOutput=False)
    wihb_d = nc.declare_dram_parameter("wihb", [128, 512], BF16, isOutput=False)
    whhf_d = nc.declare_dram_parameter("whhf", [128, 512], BF16, isOutput=False)
    whhb_d = nc.declare_dram_parameter("whhb", [128, 512], BF16, isOutput=False)
    encw_d = nc.declare_dram_parameter("encw", [128, 1024], BF16, isOutput=False)
    attnw_d = nc.declare_dram_parameter("attnw", [H, T], F32, isOutput=False)
    y_d = nc.declare_dram_parameter("y", [MS, ENC], F32, isOutput=True)

    att_stage = nc.dram_tensor("att_stage", [T, MS], BF16)

    reps = int(os.environ.get("KERNEL_REPS", "1"))
    with TileContext(nc) as tc:
      for _rep in range(reps):
        with tc.tile_pool(name="persist", bufs=1) as pp:
            ident_bf = pp.tile([128, 128], BF16, tag="ident_bf")
            make_identity(nc, ident_bf[:])
            ident_f32 = pp.tile([128, 128], F32, tag="ident_f32")
            make_identity(nc, ident_f32[:])
            zeros = pp.tile([128, MS], F32, tag="zeros")
            nc.vector.memset(zeros[:], 0.0)

            aux = pp.tile([128, 6], F32, tag="aux")
            nc.sync.dma_start(out=aux[:], in_=aux_d[:])
            maskf = pp.tile([1, MS], F32, tag="maskf")
            nc.sync.dma_start(out=maskf[:], in_=maskf_d[:])
            wpad = pp.tile([128, 2048], BF16, tag="wpad")
            nc.sync.dma_start(out=wpad[:], in_=wpad_d[:])
            wih = {}
            whh = {}
            wih["f"] = pp.tile([128, 512], BF16, tag="wihf", name="wihf_t")
            nc.sync.dma_start(out=wih["f"][:], in_=wihf_d[:])
            wih["b"] = pp.tile([128, 512], BF16, tag="wihb", name="wihb_t")
            nc.sync.dma_start(out=wih["b"][:], in_=wihb_d[:])
            whh["f"] = pp.tile([128, 512], BF16, tag="whhf", name="whhf_t")
            nc.sync.dma_start(out=whh["f"][:], in_=whhf_d[:])
            whh["b"] = pp.tile([128, 512], BF16, tag="whhb", name="whhb_t")
            nc.sync.dma_start(out=whh["b"][:], in_=whhb_d[:])
            encw = pp.tile([128, 1024], BF16, tag="encw")
            nc.sync.dma_start(out=encw[:], in_=encw_d[:])
            attnw = pp.tile([H, T], F32, tag="attnw")
            nc.sync.dma_start(out=attnw[:], in_=attnw_d[:])

            xT = pp.tile([128, 50 * MS], BF16, tag="xT")

            mx = {d: pp.tile([128, MS], BF16, tag=f"mx_{d}", name=f"mx_{d}") for d in "fb"}
            attn = {d: pp.tile([128, MS], F32, tag=f"attn_{d}", name=f"attn_{d}") for d in "fb"}
            front = {d: pp.tile([128, MS], BF16, tag=f"front_{d}", name=f"front_{d}") for d in "fb"}
            backt = {d: pp.tile([128, MS], BF16, tag=f"back_{d}", name=f"back_{d}") for d in "fb"}

            # ---------------- prelude ------------------------------------
            with tc.tile_pool(name="pre_sb", bufs=2) as sb, \
                 tc.tile_pool(name="pre_ps", bufs=1, space="PSUM") as pps:
                # lane feature transpose + embed first: the LSTM cannot
                # start until the embed's PSUM banks drain, so this is the
                # prelude critical path.
                lf_t = []
                for mc in range(4):
                    l_t = sb.tile([128, 400], BF16, tag=f"lf_in{mc}", bufs=1,
                                  name=f"lf_in{mc}")
                    nc.gpsimd.dma_start(out=l_t[:],
                                        in_=lf_d[128 * mc:128 * (mc + 1), :])
                    lf_t.append(l_t)
                lft = []
                for cb in range(4):
                    cnt = 128 if cb < 3 else 16
                    t_ps = pps.tile([128, MS], BF16, tag="pre", bufs=4)
                    for mc in range(4):
                        nc.tensor.transpose(t_ps[0:cnt, 128 * mc:128 * (mc + 1)],
                                            lf_t[mc][:, 128 * cb:128 * cb + cnt],
                                            ident_bf[:])
                    lt = sb.tile([128, MS], BF16, tag=f"lft{cb}", bufs=1)
                    if cb == 3:
                        nc.gpsimd.memset(lt[:], 0.0)
                        nc.vector.tensor_copy(lt[0:16, :], t_ps[0:16, :])
                    else:
                        nc.vector.tensor_copy(lt[:], t_ps[:])
                    lft.append(lt)

                # both chain heads need their x tiles first: fwd consumes
                # j=0,1,... while bwd consumes j=49,48,...; emit from both
                # ends and alternate the relu-drain between ACT and DVE
                emb_order = []
                for a in range(25):
                    emb_order += [a, 49 - a]
                for ji, j in enumerate(emb_order):
                    b = j // 16
                    pj = j % 16
                    x_ps = pps.tile([128, MS], F32, tag="pre", bufs=4)
                    nc.tensor.matmul(x_ps[:], wpad[:, 128 * pj:128 * (pj + 1)],
                                     lft[b][:], start=True, stop=True)
                    if ji % 2 == 0:
                        nc.scalar.activation(xT[:, MS * j:MS * (j + 1)], x_ps[:],
                                             AF.Relu)
                    else:
                        nc.vector.tensor_scalar(xT[:, MS * j:MS * (j + 1)],
                                                x_ps[:], 0.0, None, OP.max)

                # obs (256,128) -> obsT (128, 256) f32
                obsT = sb.tile([H, N_OBS], F32, tag="obsT")
                for nb in range(2):
                    o_t = sb.tile([128, H], F32, tag="obs_in")
                    nc.sync.dma_start(out=o_t[:], in_=obs_d[128 * nb:128 * (nb + 1), :])
                    tp = pps.tile([128, 128], F32, tag="obsT_ps")
                    nc.tensor.transpose(tp[:], o_t[:], ident_f32[:])
                    nc.vector.tensor_copy(obsT[:, 128 * nb:128 * (nb + 1)], tp[:])
                # scores rows (2 x (128, 100) f32), softmax over free dim
                scores = []
                for nb in range(2):
                    sc_ps = pps.tile([128, T], F32, tag="sc_ps", bufs=2)
                    nc.tensor.matmul(sc_ps[:], obsT[:, 128 * nb:128 * (nb + 1)],
                                     attnw[:], start=True, stop=True)
                    e_t = sb.tile([128, T], F32, tag="sc_relu", bufs=2)
                    nc.scalar.activation(e_t[:], sc_ps[:], AF.Relu)
                    nc.scalar.activation(e_t[:], e_t[:], AF.Exp)
                    ssum = sb.tile([128, 1], F32, tag="sc_sum", bufs=2)
                    nc.vector.tensor_reduce(ssum[:], e_t[:], mybir.AxisListType.X, OP.add)
                    rec = sb.tile([128, 1], F32, tag="sc_rec", bufs=2)
                    nc.vector.reciprocal(rec[:], ssum[:])
                    sc_t = sb.tile([128, T], F32, tag="sc_out", bufs=2)
                    nc.vector.tensor_scalar(sc_t[:], e_t[:], rec[:], None, OP.mult)
                    scores.append(sc_t)
                # one-hot gather: attT[t, m] = scores[mask[m], t]
                mask_bc = sb.tile([128, MS], F32, tag="mask_bc")
                nc.gpsimd.partition_broadcast(mask_bc[:], maskf[:])
                attT_ps = pps.tile([T, MS], F32, tag="attT_ps")
                for nb in range(2):
                    oh = sb.tile([128, MS], F32, tag="onehot", bufs=2)
                    nc.vector.tensor_scalar(oh[:], mask_bc[:], aux[:, 4 + nb:5 + nb],
                                            None, OP.is_equal)
                    nc.tensor.matmul(attT_ps[:], scores[nb][:], oh[:],
                                     start=(nb == 0), stop=(nb == 1))
                attT = sb.tile([T, MS], BF16, tag="attT")
                nc.vector.tensor_copy(attT[:], attT_ps[:])
                nc.sync.dma_start(out=att_stage[:], in_=attT[:])

            # ---------------- LSTM state init ------------------------------
            h_cur = {}
            c_cur = {}
            with tc.tile_pool(name="state", bufs=4) as st, \
                 tc.tile_pool(name="lstm_ps", bufs=1, space="PSUM") as lps, \
                 tc.tile_pool(name="stream", bufs=3) as sm:
                for di, d in enumerate("fb"):
                    # init h0/c0 on the prelude-idle Pool engine (broadcast
                    # the per-partition initial state across lanes)
                    h_cur[d] = st.tile([128, MS], BF16, tag=f"h_{d}", name=f"h0_{d}")
                    nc.gpsimd.tensor_scalar(h_cur[d][:], zeros[:],
                                            aux[:, 2 * di:2 * di + 1], None,
                                            OP.add)
                    c_cur[d] = st.tile([128, MS], F32, tag=f"c_{d}", name=f"c0_{d}")
                    nc.gpsimd.tensor_scalar(c_cur[d][:], zeros[:],
                                            aux[:, 2 * di + 1:2 * di + 2], None,
                                            OP.add)
                    nc.gpsimd.memset(attn[d][:], 0.0)

                # ---------------- the 100 bidirectional steps --------------
                # Emission is phase-interleaved across the two independent
                # chains so each engine's static order alternates f/b work:
                # while one chain's cell update runs on DVE/Pool, ACT chews
                # the other chain's gate activations instead of stalling.
                cur = {d: {} for d in "fb"}

                def emit_wx(d, s):
                    t = s if d == "f" else T - 1 - s
                    po = 64 * (t % 2)
                    xsl = xT[po:po + 64, MS * (t // 2):MS * (t // 2 + 1)]
                    wx = wih[d]
                    ifo_ps = lps.tile([128, 1536], F32, tag=f"ifo_{d}",
                                      name=f"ifo_ps_{d}")
                    g_ps = lps.tile([128, MS], F32, tag=f"g_{d}",
                                    name=f"g_ps_{d}")
                    for gi in range(3):
                        nc.tensor.matmul(ifo_ps[:, 512 * gi:512 * (gi + 1)],
                                         wx[po:po + 64, 128 * gi:128 * (gi + 1)],
                                         xsl, start=True, stop=False)
                    nc.tensor.matmul(g_ps[:], wx[po:po + 64, 384:512], xsl,
                                     start=True, stop=False)
                    cur[d]["ifo_ps"] = ifo_ps
                    cur[d]["g_ps"] = g_ps

                def emit_whh(d):
                    wh = whh[d]
                    ifo_ps = cur[d]["ifo_ps"]
                    g_ps = cur[d]["g_ps"]
                    h = h_cur[d]
                    for hv in range(2):
                        hs = slice(256 * hv, 256 * (hv + 1))
                        for gi in range(3):
                            nc.tensor.matmul(
                                ifo_ps[:, 512 * gi + 256 * hv:512 * gi + 256 * (hv + 1)],
                                wh[:, 128 * gi:128 * (gi + 1)],
                                h[:, hs], start=False, stop=(hv == 1))
                        nc.tensor.matmul(g_ps[:, hs], wh[:, 384:512], h[:, hs],
                                         start=False, stop=(hv == 1))

                def emit_sig(d):
                    sig = sm.tile([128, 1536], BF16, tag=f"sig_{d}", bufs=3,
                                  name=f"sig_{d}")
                    ifo = cur[d]["ifo_ps"]
                    nc.scalar.activation(sig[:], ifo[:], AF.Sigmoid)
                    cur[d]["sig"] = sig

                def emit_tg(d):
                    tg = sm.tile([128, MS], BF16, tag=f"tg_{d}", bufs=3,
                                 name=f"tg_{d}")
                    nc.scalar.activation(tg[:], cur[d]["g_ps"][:], AF.Tanh)
                    cur[d]["tg"] = tg

                def emit_cell(d, hv):
                    # lane-halved cell update: lo half first so the chain
                    # tail (c -> tanh -> h -> Whh) pipelines per half
                    sig = cur[d]["sig"]
                    hs = slice(256 * hv, 256 * (hv + 1))
                    if hv == 0:
                        cur[d]["mt"] = sm.tile([128, MS], BF16, tag=f"mt_{d}",
                                               bufs=2, name=f"mt_{d}")
                        cur[d]["pt"] = sm.tile([128, MS], F32, tag=f"pt_{d}",
                                               bufs=2, name=f"pt_{d}")
                        cur[d]["c_next"] = st.tile([128, MS], F32, tag=f"c_{d}",
                                                   name=f"c_new_{d}")
                    mt, pt, c_new = cur[d]["mt"], cur[d]["pt"], cur[d]["c_next"]
                    nc.vector.tensor_mul(pt[:, hs], sig[:, 256 * hv:256 * (hv + 1)],
                                         c_cur[d][:, hs])
                    nc.vector.tensor_mul(mt[:, hs], sig[:, 512 + 256 * hv:512 + 256 * (hv + 1)],
                                         cur[d]["tg"][:, hs])
                    nc.vector.tensor_add(c_new[:, hs], pt[:, hs], mt[:, hs])

                def emit_th(d, hv):
                    hs = slice(256 * hv, 256 * (hv + 1))
                    if hv == 0:
                        cur[d]["th"] = sm.tile([128, MS], BF16, tag=f"th_{d}",
                                               bufs=2, name=f"th_{d}")
                    nc.scalar.activation(cur[d]["th"][:, hs],
                                         cur[d]["c_next"][:, hs], AF.Tanh)

                def emit_h(d, hv):
                    hs = slice(256 * hv, 256 * (hv + 1))
                    if hv == 0:
                        cur[d]["h_next"] = st.tile([128, MS], BF16, tag=f"h_{d}",
                                                   name=f"h_new_{d}")
                    nc.vector.tensor_mul(cur[d]["h_next"][:, hs],
                                         cur[d]["sig"][:, 1024 + 256 * hv:1024 + 256 * (hv + 1)],
                                         cur[d]["th"][:, hs])

                def finish_step(d):
                    c_cur[d] = cur[d]["c_next"]
                    h_cur[d] = cur[d]["h_next"]

                def emit_post(d, s):
                    t = s if d == "f" else T - 1 - s
                    h_new = h_cur[d]
                    attb = sm.tile([128, MS], BF16, tag=f"attb_{d}", bufs=3,
                                   name=f"attb_{d}")
                    bc_ap = bass.AP(tensor=att_stage, offset=t * MS,
                                    ap=[[0, 128], [1, MS]])
                    nc.sync.dma_start(out=attb[:], in_=bc_ap)
                    prod = sm.tile([128, MS], F32, tag=f"prod_{d}", bufs=3,
                                   name=f"prod_{d}")
                    nc.gpsimd.tensor_mul(prod[:], h_new[:], attb[:])
                    nc.gpsimd.tensor_add(attn[d][:], attn[d][:], prod[:])
                    if s == 0:
                        nc.gpsimd.tensor_copy(mx[d][:], h_new[:])
                        dst = front[d] if d == "f" else backt[d]
                        nc.vector.tensor_copy(dst[:], h_new[:])
                    else:
                        nc.vector.tensor_max(mx[d][:], mx[d][:], h_new[:])
                        if s == T - 1:
                            dst = backt[d] if d == "f" else front[d]
                            nc.vector.tensor_copy(dst[:], h_new[:])

                def emit_step_solo(d, s):
                    emit_wx(d, s)
                    emit_whh(d)
                    emit_sig(d)
                    emit_tg(d)
                    for hv in range(2):
                        emit_cell(d, hv)
                        emit_th(d, hv)
                        emit_h(d, hv)
                    finish_step(d)
                    emit_post(d, s)

                emit_step_solo("f", 0)
                for s in range(1, T):
                    emit_wx("f", s)
                    emit_wx("b", s - 1)
                    emit_whh("f")
                    emit_whh("b")
                    for d in ("f", "b"):
                        emit_sig(d)
                        emit_tg(d)
                    for d in ("f", "b"):
                        for hv in range(2):
                            emit_cell(d, hv)
                            emit_th(d, hv)
                            emit_h(d, hv)
                        finish_step(d)
                    emit_post("f", s)
                    emit_post("b", s - 1)
                emit_step_solo("b", T - 1)

            # ---------------- final encoder --------------------------------
            with tc.tile_pool(name="fin_sb", bufs=2) as fs, \
                 tc.tile_pool(name="fin_ps", bufs=1, space="PSUM") as fps:
                attnb = {}
                for d in "fb":
                    attnb[d] = fs.tile([128, MS], BF16, tag=f"attnb_{d}", bufs=1, name=f"attnb_{d}")
                    nc.vector.tensor_copy(attnb[d][:], attn[d][:])
                blocks = [front["f"], front["b"], backt["f"], backt["b"],
                          mx["f"], mx["b"], attnb["f"], attnb["b"]]
                o_ps = fps.tile([128, MS], F32, tag="out_ps")
                for bi, blk in enumerate(blocks):
                    nc.tensor.matmul(o_ps[:], encw[:, 128 * bi:128 * (bi + 1)],
                                     blk[:], start=(bi == 0), stop=(bi == 7))
                outT = fs.tile([128, MS], F32, tag="outT", bufs=1)
                nc.scalar.activation(outT[:], o_ps[:], AF.Relu)
                for mc in range(4):
                    t_ps = fps.tile([128, 128], F32, tag="otr_ps", bufs=2)
                    nc.tensor.transpose(t_ps[:], outT[:, 128 * mc:128 * (mc + 1)],
                                        ident_f32[:])
                    o_sb = fs.tile([128, 128], F32, tag="o_sb", bufs=2)
                    nc.vector.tensor_copy(o_sb[:], t_ps[:])
                    nc.sync.dma_start(out=y_d[128 * mc:128 * (mc + 1), :],
                                      in_=o_sb[:])

    nc.compile()
    return nc


def _prep_host(inputs):
    """Build per-core input maps (numpy only; weights replicated)."""
    lf = np.ascontiguousarray(inputs["lane_features"][:, 200:600], np.float32)
    obs = np.ascontiguousarray(np.asarray(inputs["obs_encoding"], np.float32))
    mask = np.asarray(inputs["same_obs_mask"]).reshape(-1).astype(np.float32)

    def reorder(w):  # pytorch gate order [i,f,g,o] -> ours [f,i,o,g]
        w = np.asarray(w, np.float32)
        return np.concatenate([w[:, H:2 * H], w[:, 0:H], w[:, 3 * H:4 * H],
                               w[:, 2 * H:3 * H]], axis=1)

    wihf = reorder(inputs["Wih_f"])
    wihb = reorder(inputs["Wih_b"])
    wih_f2 = np.concatenate([wihf, wihf], axis=0).astype(nbf16)
    wih_b2 = np.concatenate([wihb, wihb], axis=0).astype(nbf16)
    whhf = reorder(inputs["Whh_f"]).astype(nbf16)
    whhb = reorder(inputs["Whh_b"]).astype(nbf16)

    embw = np.asarray(inputs["embed_W"], np.float32)  # (4, 64)
    wpad = np.zeros((128, 2048), np.float32)
    for tau in range(32):
        j, half = tau // 2, tau % 2
        wpad[4 * tau:4 * tau + 4, 128 * j + 64 * half:128 * j + 64 * half + 64] = embw
    wpad = wpad.astype(nbf16)

    encw = np.asarray(inputs["enc_W"], np.float32)  # (1024, 128)
    encw_t = np.zeros((128, 1024), np.float32)
    for b in range(8):
        encw_t[:, 128 * b:128 * (b + 1)] = encw[128 * b:128 * (b + 1), :]
    encw_t = encw_t.astype(nbf16)

    h0 = np.asarray(inputs["h0"], np.float32)
    c0 = np.asarray(inputs["c0"], np.float32)
    aux = np.zeros((128, 6), np.float32)
    aux[:, 0] = h0[0, 0]
    aux[:, 1] = c0[0, 0]
    aux[:, 2] = h0[1, 0]
    aux[:, 3] = c0[1, 0]
    aux[:, 4] = np.arange(128)
    aux[:, 5] = 128 + np.arange(128)

    attnw = np.asarray(inputs["attn_W"], np.float32)

    shared = {
        "obs": obs, "aux": aux, "wpad": wpad,
        "wihf": wih_f2, "wihb": wih_b2, "whhf": whhf, "whhb": whhb,
        "encw": encw_t, "attnw": attnw,
    }
    in_maps = []
    for c in range(N_CORES):
        m = dict(shared)
        m["lf"] = lf[MS * c:MS * (c + 1)]
        m["maskf"] = mask[MS * c:MS * (c + 1)].reshape(1, MS)
        in_maps.append(m)
    return in_maps


_CACHE = {}


def _get_runner():
    """Compile once and build a cached jitted 8-core executor."""
    if "run" in _CACHE:
        return _CACHE["run"]

    nc = _build_program()

    import jax
    from jax.sharding import Mesh, PartitionSpec
    from jax.experimental.shard_map import shard_map
    from concourse import bass2jax

    bass2jax.install_neuronx_cc_hook()

    partition_name = (nc.partition_id_tensor.name
                      if nc.partition_id_tensor else None)
    in_names, out_names, out_avals, zero_outs = [], [], [], []
    for alloc in nc.m.functions[0].allocations:
        if not isinstance(alloc, mybir.MemoryLocationSet):
            continue
        name = alloc.memorylocations[0].name
        if alloc.kind == "ExternalInput":
            if name != partition_name:
                in_names.append(name)
        elif alloc.kind == "ExternalOutput":
            shape = tuple(alloc.tensor_shape)
            dtype = mybir.dt.np(alloc.dtype)
            out_names.append(name)
            out_avals.append(jax.core.ShapedArray(shape, dtype))
            zero_outs.append(np.zeros(shape, dtype))
    n_params = len(in_names)
    n_outs = len(out_avals)
    all_names = list(in_names) + list(out_names)
    if partition_name is not None:
        all_names.append(partition_name)

    def _body(*args):
        operands = list(args)
        if partition_name is not None:
            operands.append(bass2jax.partition_id_tensor())
        outs = bass2jax._bass_exec_p.bind(
            *operands,
            out_avals=tuple(out_avals),
            in_names=tuple(all_names),
            out_names=tuple(out_names),
            lowering_input_output_aliases=(),
            sim_require_finite=True,
            sim_require_nnan=True,
            nc=nc,
        )
        return tuple(outs)

    devices = jax.devices()[:N_CORES]
    mesh = Mesh(np.asarray(devices), ("core",))
    donate = tuple(range(n_params, n_params + n_outs))
    sharded = jax.jit(
        shard_map(_body, mesh=mesh,
                  in_specs=(PartitionSpec("core"),) * (n_params + n_outs),
                  out_specs=(PartitionSpec("core"),) * n_outs,
                  check_rep=False),
        donate_argnums=donate, keep_unused=True)

    def run(in_maps):
        concat_in = [np.concatenate([np.asarray(in_maps[c][nm])
                                     for c in range(N_CORES)], axis=0)
                     for nm in in_names]
        concat_zeros = [np.zeros((N_CORES * z.shape[0], *z.shape[1:]), z.dtype)
                        for z in zero_outs]
        out_arrs = sharded(*concat_in, *concat_zeros)
        return [{nm: np.asarray(out_arrs[i]).reshape(N_CORES,
                                                     *out_avals[i].shape)[c]
                 for i, nm in enumerate(out_names)}
                for c in range(N_CORES)]

    _CACHE.update(run=run, nc=nc, in_names=in_names, out_names=out_names,
                  out_avals=out_avals, zero_outs=zero_outs,
                  partition_name=partition_name, all_names=all_names,
                  sharded=sharded)
    return run


def kernel(**inputs) -> np.ndarray:
    in_maps = _prep_host(inputs)
    run = _get_runner()
    results = run(in_maps)
    return np.concatenate([results[c]["y"] for c in range(N_CORES)], axis=0)


# revision 42
# speedup vs baseline: 1.4612x; 1.4612x over previous
"""Trainium2 Bass kernel for the AttentionalLaneLSTM problem.

Full-input contract: kernel(**inputs) takes the complete (unsharded) numpy
inputs and returns the full (4096, 128) float32 output. Internally the lane
dimension M=4096 is sharded 8 ways (512 lanes per NeuronCore); all small
weights and the obstacle encoding are replicated so each core computes its
own attention table and gathers its rows locally.

Per-core dataflow (everything feature-major: features on SBUF partitions,
lanes on the free dim so the LSTM recurrence needs no transposes):
  1. scores = softmax(relu(obs @ attn_W)) computed on-chip (256x100).
  2. att^T = scores^T gathered per lane via a one-hot matmul
     (att^T[t,m] = scores[mask[m], t]), staged to DRAM for per-step
     partition-broadcast loads.
  3. lane_features tail (512,400) is transposed on the PE, embedded for two
     timesteps per matmul via a padded block-diagonal embed weight, relu'd
     into a packed x^T buffer (128, 25600) bf16 (even t rows 0:64, odd 64:128).
  4. 100 bidirectional LSTM steps, fwd/bwd chains phase-interleaved.
     Gates accumulate x@Wih + h@Whh in PSUM (8 banks = 2 dirs x [f|i|o]+g),
     sigmoid/tanh on ScalarE, cell update + running max on VectorE with a
     lane-halved tail (c/tanh/h in 256-lane halves so the recurrence tail
     pipelines), attention product + accumulate on GpSimd, per-step
     attention rows broadcast-loaded from a DRAM stage via stride-0 DMA.
  5. Final encoder: 8 accumulating matmuls over [front|back|max|attn]
     feature blocks, relu, PE transpose back to lane-major, DMA out.
"""

import os
import sys

if "/opt/trn_rl_repo" not in sys.path:
    sys.path.insert(0, "/opt/trn_rl_repo")

import numpy as np
import ml_dtypes

import concourse.bass as bass
import concourse.bacc as bacc
import concourse.mybir as mybir
from concourse.tile import TileContext
from concourse.masks import make_identity

BF16 = mybir.dt.bfloat16
F32 = mybir.dt.float32
AF = mybir.ActivationFunctionType
OP = mybir.AluOpType
nbf16 = ml_dtypes.bfloat16

N_CORES = 8
M, N_OBS, EMB, H, ENC, T = 4096, 256, 64, 128, 128, 100
MS = M // N_CORES  # 512 lanes per core


def _build_program():
    nc = bacc.Bacc("TRN2", target_bir_lowering=False, debug=False,
                   num_devices=N_CORES)

    lf_d = nc.declare_dram_parameter("lf", [MS, 400], F32, isOutput=False)
    obs_d = nc.declare_dram_parameter("obs", [N_OBS, H], F32, isOutput=False)
    maskf_d = nc.declare_dram_parameter("maskf", [1, MS], F32, isOutput=False)
    aux_d = nc.declare_dram_parameter("aux", [128, 6], F32, isOutput=False)
    wpad_d = nc.declare_dram_parameter("wpad", [128, 2048], BF16, isOutput=False)
    wihf_d = nc.declare_dram_parameter("wihf", [128, 512], BF16, isOutput=False)
    wihb_d = nc.declare_dram_parameter("wihb", [128, 512], BF16, isOutput=False)
    whhf_d = nc.declare_dram_parameter("whhf", [128, 512], BF16, isOutput=False)
    whhb_d = nc.declare_dram_parameter("whhb", [128, 512], BF16, isOutput=False)
    encw_d = nc.declare_dram_parameter("encw", [128, 1024], BF16, isOutput=False)
    attnw_d = nc.declare_dram_parameter("attnw", [H, T], F32, isOutput=False)
    y_d = nc.declare_dram_parameter("y", [MS, ENC], F32, isOutput=True)

    att_stage = nc.dram_tensor("att_stage", [T, MS], BF16)

    reps = int(os.environ.get("KERNEL_REPS", "1"))
    with TileContext(nc) as tc:
      for _rep in range(reps):
        with tc.tile_pool(name="persist", bufs=1) as pp:
            ident_bf = pp.tile([128, 128], BF16, tag="ident_bf")
            make_identity(nc, ident_bf[:])
            ident_f32 = pp.tile([128, 128], F32, tag="ident_f32")
            make_identity(nc, ident_f32[:])
            zeros = pp.tile([128, MS], F32, tag="zeros")
            nc.vector.memset(zeros[:], 0.0)

            aux = pp.tile([128, 6], F32, tag="aux")
            nc.sync.dma_start(out=aux[:], in_=aux_d[:])
            maskf = pp.tile([1, MS], F32, tag="maskf")
            nc.sync.dma_start(out=maskf[:], in_=maskf_d[:])
            wpad = pp.tile([128, 2048], BF16, tag="wpad")
            nc.sync.dma_start(out=wpad[:], in_=wpad_d[:])
            wih = {}
            whh = {}
            wih["f"] = pp.tile([128, 512], BF16, tag="wihf", name="wihf_t")
            nc.sync.dma_start(out=wih["f"][:], in_=wihf_d[:])
            wih["b"] = pp.tile([128, 512], BF16, tag="wihb", name="wihb_t")
            nc.sync.dma_start(out=wih["b"][:], in_=wihb_d[:])
            whh["f"] = pp.tile([128, 512], BF16, tag="whhf", name="whhf_t")
            nc.sync.dma_start(out=whh["f"][:], in_=whhf_d[:])
            whh["b"] = pp.tile([128, 512], BF16, tag="whhb", name="whhb_t")
            nc.sync.dma_start(out=whh["b"][:], in_=whhb_d[:])
            encw = pp.tile([128, 1024], BF16, tag="encw")
            nc.sync.dma_start(out=encw[:], in_=encw_d[:])
            attnw = pp.tile([H, T], F32, tag="attnw")
            nc.sync.dma_start(out=attnw[:], in_=attnw_d[:])

            xT = pp.tile([128, 50 * MS], BF16, tag="xT")

            mx = {d: pp.tile([128, MS], BF16, tag=f"mx_{d}", name=f"mx_{d}") for d in "fb"}
            attn = {d: pp.tile([128, MS], F32, tag=f"attn_{d}", name=f"attn_{d}") for d in "fb"}
            front = {d: pp.tile([128, MS], BF16, tag=f"front_{d}", name=f"front_{d}") for d in "fb"}
            backt = {d: pp.tile([128, MS], BF16, tag=f"back_{d}", name=f"back_{d}") for d in "fb"}

            # ---------------- prelude ------------------------------------
            with tc.tile_pool(name="pre_sb", bufs=2) as sb, \
                 tc.tile_pool(name="pre_ps", bufs=1, space="PSUM") as pps:
                # lane feature transpose + embed first: the LSTM cannot
                # start until the embed's PSUM banks drain, so this is the
                # prelude critical path.
                lf_t = []
                for mc in range(4):
                    l_t = sb.tile([128, 400], BF16, tag=f"lf_in{mc}", bufs=1,
                                  name=f"lf_in{mc}")
                    nc.gpsimd.dma_start(out=l_t[:],
                                        in_=lf_d[128 * mc:128 * (mc + 1), :])
                    lf_t.append(l_t)
                lft = []
                for cb in range(4):
                    cnt = 128 if cb < 3 else 16
                    t_ps = pps.tile([128, MS], BF16, tag="pre", bufs=4)
                    for mc in range(4):
                        nc.tensor.transpose(t_ps[0:cnt, 128 * mc:128 * (mc + 1)],
                                            lf_t[mc][:, 128 * cb:128 * cb + cnt],
                                            ident_bf[:])
                    lt = pp.tile([128, MS], BF16, tag=f"lft{cb}", name=f"lft{cb}")
                    if cb == 3:
                        nc.gpsimd.memset(lt[:], 0.0)
                        nc.vector.tensor_copy(lt[0:16, :], t_ps[0:16, :])
                    else:
                        nc.vector.tensor_copy(lt[:], t_ps[:])
                    lft.append(lt)

                # both chain heads need their x tiles first: fwd consumes
                # j=0,1,... while bwd consumes j=49,48,...; emit from both
                # ends and alternate the relu-drain between ACT and DVE
                emb_order = []
                for a in range(6):
                    emb_order += [a, 49 - a]
                for ji, j in enumerate(emb_order):
                    b = j // 16
                    pj = j % 16
                    x_ps = pps.tile([128, MS], F32, tag="pre", bufs=4)
                    nc.tensor.matmul(x_ps[:], wpad[:, 128 * pj:128 * (pj + 1)],
                                     lft[b][:], start=True, stop=True)
                    if ji % 2 == 0:
                        nc.scalar.activation(xT[:, MS * j:MS * (j + 1)], x_ps[:],
                                             AF.Relu)
                    else:
                        nc.vector.tensor_scalar(xT[:, MS * j:MS * (j + 1)],
                                                x_ps[:], 0.0, None, OP.max)

                # obs (256,128) -> obsT (128, 256) f32
                obsT = sb.tile([H, N_OBS], F32, tag="obsT")
                for nb in range(2):
                    o_t = sb.tile([128, H], F32, tag="obs_in")
                    nc.sync.dma_start(out=o_t[:], in_=obs_d[128 * nb:128 * (nb + 1), :])
                    tp = pps.tile([128, 128], F32, tag="obsT_ps")
                    nc.tensor.transpose(tp[:], o_t[:], ident_f32[:])
                    nc.vector.tensor_copy(obsT[:, 128 * nb:128 * (nb + 1)], tp[:])
                # scores rows (2 x (128, 100) f32), softmax over free dim
                scores = []
                for nb in range(2):
                    sc_ps = pps.tile([128, T], F32, tag="sc_ps", bufs=2)
                    nc.tensor.matmul(sc_ps[:], obsT[:, 128 * nb:128 * (nb + 1)],
                                     attnw[:], start=True, stop=True)
                    e_t = sb.tile([128, T], F32, tag="sc_relu", bufs=2)
                    nc.scalar.activation(e_t[:], sc_ps[:], AF.Relu)
                    nc.scalar.activation(e_t[:], e_t[:], AF.Exp)
                    ssum = sb.tile([128, 1], F32, tag="sc_sum", bufs=2)
                    nc.vector.tensor_reduce(ssum[:], e_t[:], mybir.AxisListType.X, OP.add)
                    rec = sb.tile([128, 1], F32, tag="sc_rec", bufs=2)
                    nc.vector.reciprocal(rec[:], ssum[:])
                    sc_t = sb.tile([128, T], F32, tag="sc_out", bufs=2)
                    nc.vector.tensor_scalar(sc_t[:], e_t[:], rec[:], None, OP.mult)
                    scores.append(sc_t)
                # one-hot gather: attT[t, m] = scores[mask[m], t]
                mask_bc = sb.tile([128, MS], F32, tag="mask_bc")
                nc.gpsimd.partition_broadcast(mask_bc[:], maskf[:])
                attT_ps = pps.tile([T, MS], F32, tag="attT_ps")
                for nb in range(2):
                    oh = sb.tile([128, MS], F32, tag="onehot", bufs=2)
                    nc.vector.tensor_scalar(oh[:], mask_bc[:], aux[:, 4 + nb:5 + nb],
                                            None, OP.is_equal)
                    nc.tensor.matmul(attT_ps[:], scores[nb][:], oh[:],
                                     start=(nb == 0), stop=(nb == 1))
                attT = sb.tile([T, MS], BF16, tag="attT")
                nc.vector.tensor_copy(attT[:], attT_ps[:])
                nc.sync.dma_start(out=att_stage[:], in_=attT[:])

            # ---------------- LSTM state init ------------------------------
            h_cur = {}
            c_cur = {}
            with tc.tile_pool(name="state", bufs=4) as st, \
                 tc.tile_pool(name="lstm_ps", bufs=1, space="PSUM") as lps, \
                 tc.tile_pool(name="stream", bufs=3) as sm:
                for di, d in enumerate("fb"):
                    # init h0/c0 on the prelude-idle Pool engine (broadcast
                    # the per-partition initial state across lanes)
                    h_cur[d] = st.tile([128, MS], BF16, tag=f"h_{d}", name=f"h0_{d}")
                    nc.gpsimd.tensor_scalar(h_cur[d][:], zeros[:],
                                            aux[:, 2 * di:2 * di + 1], None,
                                            OP.add)
                    c_cur[d] = st.tile([128, MS], F32, tag=f"c_{d}", name=f"c0_{d}")
                    nc.gpsimd.tensor_scalar(c_cur[d][:], zeros[:],
                                            aux[:, 2 * di + 1:2 * di + 2], None,
                                            OP.add)
                    nc.gpsimd.memset(attn[d][:], 0.0)

                # ---------------- the 100 bidirectional steps --------------
                # Emission is phase-interleaved across the two independent
                # chains so each engine's static order alternates f/b work:
                # while one chain's cell update runs on DVE/Pool, ACT chews
                # the other chain's gate activations instead of stalling.
                cur = {d: {} for d in "fb"}

                def emit_wx(d, s):
                    t = s if d == "f" else T - 1 - s
                    po = 64 * (t % 2)
                    xsl = xT[po:po + 64, MS * (t // 2):MS * (t // 2 + 1)]
                    wx = wih[d]
                    ifo_ps = lps.tile([128, 1536], F32, tag=f"ifo_{d}",
                                      name=f"ifo_ps_{d}")
                    g_ps = lps.tile([128, MS], F32, tag=f"g_{d}",
                                    name=f"g_ps_{d}")
                    for gi in range(3):
                        nc.tensor.matmul(ifo_ps[:, 512 * gi:512 * (gi + 1)],
                                         wx[po:po + 64, 128 * gi:128 * (gi + 1)],
                                         xsl, start=True, stop=False)
                    nc.tensor.matmul(g_ps[:], wx[po:po + 64, 384:512], xsl,
                                     start=True, stop=False)
                    cur[d]["ifo_ps"] = ifo_ps
                    cur[d]["g_ps"] = g_ps

                def emit_whh(d):
                    wh = whh[d]
                    ifo_ps = cur[d]["ifo_ps"]
                    g_ps = cur[d]["g_ps"]
                    h = h_cur[d]
                    for hv in range(2):
                        hs = slice(256 * hv, 256 * (hv + 1))
                        for gi in range(3):
                            nc.tensor.matmul(
                                ifo_ps[:, 512 * gi + 256 * hv:512 * gi + 256 * (hv + 1)],
                                wh[:, 128 * gi:128 * (gi + 1)],
                                h[:, hs], start=False, stop=(hv == 1))
                        nc.tensor.matmul(g_ps[:, hs], wh[:, 384:512], h[:, hs],
                                         start=False, stop=(hv == 1))

                def emit_sig(d):
                    sig = sm.tile([128, 1536], BF16, tag=f"sig_{d}", bufs=3,
                                  name=f"sig_{d}")
                    ifo = cur[d]["ifo_ps"]
                    nc.scalar.activation(sig[:], ifo[:], AF.Sigmoid)
                    cur[d]["sig"] = sig

                def emit_tg(d):
                    tg = sm.tile([128, MS], BF16, tag=f"tg_{d}", bufs=3,
                                 name=f"tg_{d}")
                    nc.scalar.activation(tg[:], cur[d]["g_ps"][:], AF.Tanh)
                    cur[d]["tg"] = tg

                def emit_cell(d, hv):
                    # lane-halved cell update: lo half first so the chain
                    # tail (c -> tanh -> h -> Whh) pipelines per half
                    sig = cur[d]["sig"]
                    hs = slice(256 * hv, 256 * (hv + 1))
                    if hv == 0:
                        cur[d]["mt"] = sm.tile([128, MS], BF16, tag=f"mt_{d}",
                                               bufs=2, name=f"mt_{d}")
                        cur[d]["pt"] = sm.tile([128, MS], F32, tag=f"pt_{d}",
                                               bufs=2, name=f"pt_{d}")
                        cur[d]["c_next"] = st.tile([128, MS], F32, tag=f"c_{d}",
                                                   name=f"c_new_{d}")
                    mt, pt, c_new = cur[d]["mt"], cur[d]["pt"], cur[d]["c_next"]
                    nc.vector.tensor_mul(pt[:, hs], sig[:, 256 * hv:256 * (hv + 1)],
                                         c_cur[d][:, hs])
                    nc.vector.tensor_mul(mt[:, hs], sig[:, 512 + 256 * hv:512 + 256 * (hv + 1)],
                                         cur[d]["tg"][:, hs])
                    nc.vector.tensor_add(c_new[:, hs], pt[:, hs], mt[:, hs])

                def emit_th(d, hv):
                    hs = slice(256 * hv, 256 * (hv + 1))
                    if hv == 0:
                        cur[d]["th"] = sm.tile([128, MS], BF16, tag=f"th_{d}",
                                               bufs=2, name=f"th_{d}")
                    nc.scalar.activation(cur[d]["th"][:, hs],
                                         cur[d]["c_next"][:, hs], AF.Tanh)

                def emit_h(d, hv):
                    hs = slice(256 * hv, 256 * (hv + 1))
                    if hv == 0:
                        cur[d]["h_next"] = st.tile([128, MS], BF16, tag=f"h_{d}",
                                                   name=f"h_new_{d}")
                    nc.vector.tensor_mul(cur[d]["h_next"][:, hs],
                                         cur[d]["sig"][:, 1024 + 256 * hv:1024 + 256 * (hv + 1)],
                                         cur[d]["th"][:, hs])

                def finish_step(d):
                    c_cur[d] = cur[d]["c_next"]
                    h_cur[d] = cur[d]["h_next"]

                def emit_post(d, s):
                    t = s if d == "f" else T - 1 - s
                    h_new = h_cur[d]
                    attb = sm.tile([128, MS], BF16, tag=f"attb_{d}", bufs=3,
                                   name=f"attb_{d}")
                    bc_ap = bass.AP(tensor=att_stage, offset=t * MS,
                                    ap=[[0, 128], [1, MS]])
                    nc.sync.dma_start(out=attb[:], in_=bc_ap)
                    prod = sm.tile([128, MS], F32, tag=f"prod_{d}", bufs=3,
                                   name=f"prod_{d}")
                    nc.gpsimd.tensor_mul(prod[:], h_new[:], attb[:])
                    nc.gpsimd.tensor_add(attn[d][:], attn[d][:], prod[:])
                    if s == 0:
                        nc.gpsimd.tensor_copy(mx[d][:], h_new[:])
                        dst = front[d] if d == "f" else backt[d]
                        nc.vector.tensor_copy(dst[:], h_new[:])
                    else:
                        nc.vector.tensor_max(mx[d][:], mx[d][:], h_new[:])
                        if s == T - 1:
                            dst = backt[d] if d == "f" else front[d]
                            nc.vector.tensor_copy(dst[:], h_new[:])

                def emit_step_solo(d, s):
                    emit_wx(d, s)
                    emit_whh(d)
                    emit_sig(d)
                    emit_tg(d)
                    for hv in range(2):
                        emit_cell(d, hv)
                        emit_th(d, hv)
                        emit_h(d, hv)
                    finish_step(d)
                    emit_post(d, s)

                def emit_embed_fill(j, use_act, tag):
                    # stream a deferred embed matmul through an idle slot of
                    # the LSTM's g-gate PSUM rotation (the bank is free
                    # between the tanh(g) read and the next step's Wx matmul)
                    b = j // 16
                    pj = j % 16
                    e_ps = lps.tile([128, MS], F32, tag=tag, name=f"e_ps_{tag}")
                    nc.tensor.matmul(e_ps[:], wpad[:, 128 * pj:128 * (pj + 1)],
                                     lft[b][:], start=True, stop=True)
                    if use_act:
                        nc.scalar.activation(xT[:, MS * j:MS * (j + 1)],
                                             e_ps[:], AF.Relu)
                    else:
                        nc.vector.tensor_scalar(xT[:, MS * j:MS * (j + 1)],
                                                e_ps[:], 0.0, None, OP.max)

                emit_step_solo("f", 0)
                for s in range(1, T):
                    emit_wx("f", s)
                    emit_wx("b", s - 1)
                    emit_whh("f")
                    emit_whh("b")
                    for d in ("f", "b"):
                        emit_sig(d)
                        emit_tg(d)
                    for d in ("f", "b"):
                        for hv in range(2):
                            emit_cell(d, hv)
                            emit_th(d, hv)
                            emit_h(d, hv)
                        finish_step(d)
                    emit_post("f", s)
                    emit_post("b", s - 1)
                    if 1 <= s <= 20:
                        emit_embed_fill(5 + s, False, "g_f")
                    if 1 <= s <= 18:
                        emit_embed_fill(44 - s, False, "g_b")
                emit_step_solo("b", T - 1)

            # ---------------- final encoder --------------------------------
            with tc.tile_pool(name="fin_sb", bufs=2) as fs, \
                 tc.tile_pool(name="fin_ps", bufs=1, space="PSUM") as fps:
                attnb = {}
                for d in "fb":
                    attnb[d] = fs.tile([128, MS], BF16, tag=f"attnb_{d}", bufs=1, name=f"attnb_{d}")
                    nc.vector.tensor_copy(attnb[d][:], attn[d][:])
                blocks = [front["f"], front["b"], backt["f"], backt["b"],
                          mx["f"], mx["b"], attnb["f"], attnb["b"]]
                o_ps = fps.tile([128, MS], F32, tag="out_ps")
                for bi, blk in enumerate(blocks):
                    nc.tensor.matmul(o_ps[:], encw[:, 128 * bi:128 * (bi + 1)],
                                     blk[:], start=(bi == 0), stop=(bi == 7))
                outT = fs.tile([128, MS], F32, tag="outT", bufs=1)
                nc.scalar.activation(outT[:], o_ps[:], AF.Relu)
                for mc in range(4):
                    t_ps = fps.tile([128, 128], F32, tag="otr_ps", bufs=2)
                    nc.tensor.transpose(t_ps[:], outT[:, 128 * mc:128 * (mc + 1)],
                                        ident_f32[:])
                    o_sb = fs.tile([128, 128], F32, tag="o_sb", bufs=2)
                    nc.vector.tensor_copy(o_sb[:], t_ps[:])
                    nc.sync.dma_start(out=y_d[128 * mc:128 * (mc + 1), :],
                                      in_=o_sb[:])

    nc.compile()
    return nc


def _prep_host(inputs):
    """Build per-core input maps (numpy only; weights replicated)."""
    lf = np.ascontiguousarray(inputs["lane_features"][:, 200:600], np.float32)
    obs = np.ascontiguousarray(np.asarray(inputs["obs_encoding"], np.float32))
    mask = np.asarray(inputs["same_obs_mask"]).reshape(-1).astype(np.float32)

    def reorder(w):  # pytorch gate order [i,f,g,o] -> ours [f,i,o,g]
        w = np.asarray(w, np.float32)
        return np.concatenate([w[:, H:2 * H], w[:, 0:H], w[:, 3 * H:4 * H],
                               w[:, 2 * H:3 * H]], axis=1)

    wihf = reorder(inputs["Wih_f"])
    wihb = reorder(inputs["Wih_b"])
    wih_f2 = np.concatenate([wihf, wihf], axis=0).astype(nbf16)
    wih_b2 = np.concatenate([wihb, wihb], axis=0).astype(nbf16)
    whhf = reorder(inputs["Whh_f"]).astype(nbf16)
    whhb = reorder(inputs["Whh_b"]).astype(nbf16)

    embw = np.asarray(inputs["embed_W"], np.float32)  # (4, 64)
    wpad = np.zeros((128, 2048), np.float32)
    for tau in range(32):
        j, half = tau // 2, tau % 2
        wpad[4 * tau:4 * tau + 4, 128 * j + 64 * half:128 * j + 64 * half + 64] = embw
    wpad = wpad.astype(nbf16)

    encw = np.asarray(inputs["enc_W"], np.float32)  # (1024, 128)
    encw_t = np.zeros((128, 1024), np.float32)
    for b in range(8):
        encw_t[:, 128 * b:128 * (b + 1)] = encw[128 * b:128 * (b + 1), :]
    encw_t = encw_t.astype(nbf16)

    h0 = np.asarray(inputs["h0"], np.float32)
    c0 = np.asarray(inputs["c0"], np.float32)
    aux = np.zeros((128, 6), np.float32)
    aux[:, 0] = h0[0, 0]
    aux[:, 1] = c0[0, 0]
    aux[:, 2] = h0[1, 0]
    aux[:, 3] = c0[1, 0]
    aux[:, 4] = np.arange(128)
    aux[:, 5] = 128 + np.arange(128)

    attnw = np.asarray(inputs["attn_W"], np.float32)

    shared = {
        "obs": obs, "aux": aux, "wpad": wpad,
        "wihf": wih_f2, "wihb": wih_b2, "whhf": whhf, "whhb": whhb,
        "encw": encw_t, "attnw": attnw,
    }
    in_maps = []
    for c in range(N_CORES):
        m = dict(shared)
        m["lf"] = lf[MS * c:MS * (c + 1)]
        m["maskf"] = mask[MS * c:MS * (c + 1)].reshape(1, MS)
        in_maps.append(m)
    return in_maps


_CACHE = {}


def _get_runner():
    """Compile once and build a cached jitted 8-core executor."""
    if "run" in _CACHE:
        return _CACHE["run"]

    nc = _build_program()

    import jax
    from jax.sharding import Mesh, PartitionSpec
    from jax.experimental.shard_map import shard_map
    from concourse import bass2jax

    bass2jax.install_neuronx_cc_hook()

    partition_name = (nc.partition_id_tensor.name
                      if nc.partition_id_tensor else None)
    in_names, out_names, out_avals, zero_outs = [], [], [], []
    for alloc in nc.m.functions[0].allocations:
        if not isinstance(alloc, mybir.MemoryLocationSet):
            continue
        name = alloc.memorylocations[0].name
        if alloc.kind == "ExternalInput":
            if name != partition_name:
                in_names.append(name)
        elif alloc.kind == "ExternalOutput":
            shape = tuple(alloc.tensor_shape)
            dtype = mybir.dt.np(alloc.dtype)
            out_names.append(name)
            out_avals.append(jax.core.ShapedArray(shape, dtype))
            zero_outs.append(np.zeros(shape, dtype))
    n_params = len(in_names)
    n_outs = len(out_avals)
    all_names = list(in_names) + list(out_names)
    if partition_name is not None:
        all_names.append(partition_name)

    def _body(*args):
        operands = list(args)
        if partition_name is not None:
            operands.append(bass2jax.partition_id_tensor())
        outs = bass2jax._bass_exec_p.bind(
            *operands,
            out_avals=tuple(out_avals),
            in_names=tuple(all_names),
            out_names=tuple(out_names),
            lowering_input_output_aliases=(),
            sim_require_finite=True,
            sim_require_nnan=True,
            nc=nc,
        )
        return tuple(outs)

    devices = jax.devices()[:N_CORES]
    mesh = Mesh(np.asarray(devices), ("core",))
    donate = tuple(range(n_params, n_params + n_outs))
    sharded = jax.jit(
        shard_map(_body, mesh=mesh,
                  in_specs=(PartitionSpec("core"),) * (n_params + n_outs),
                  out_specs=(PartitionSpec("core"),) * n_outs,
                  check_rep=False),
        donate_argnums=donate, keep_unused=True)

    def run(in_maps):
        concat_in = [np.concatenate([np.asarray(in_maps[c][nm])
                                     for c in range(N_CORES)], axis=0)
                     for nm in in_names]
        concat_zeros = [np.zeros((N_CORES * z.shape[0], *z.shape[1:]), z.dtype)
                        for z in zero_outs]
        out_arrs = sharded(*concat_in, *concat_zeros)
        return [{nm: np.asarray(out_arrs[i]).reshape(N_CORES,
                                                     *out_avals[i].shape)[c]
                 for i, nm in enumerate(out_names)}
                for c in range(N_CORES)]

    _CACHE.update(run=run, nc=nc, in_names=in_names, out_names=out_names,
                  out_avals=out_avals, zero_outs=zero_outs,
                  partition_name=partition_name, all_names=all_names,
                  sharded=sharded)
    return run


def kernel(**inputs) -> np.ndarray:
    in_maps = _prep_host(inputs)
    run = _get_runner()
    results = run(in_maps)
    return np.concatenate([results[c]["y"] for c in range(N_CORES)], axis=0)
